# revision 1
# baseline (speedup 1.0000x reference)
"""Trainium2 Bass kernel for y = inputs @ weights.T + bias.

Shapes: inputs [8192, 4096] f32, weights [4096, 4096] f32, bias [4096] f32,
output [8192, 4096] f32.

Strategy:
- Data-parallel across 8 NeuronCores: each core computes 1024 rows of the
  output; weights/bias are replicated.
- Host pre-transposes inputs and weights to K-major layout so the device
  kernel needs no on-chip transposes (fp32 has no DMA-transpose path).
- Matmuls run in float32r (TF32-like, full PE rate at moving-dim >= 256,
  ~1.5e-4 rel err at K=4096) accumulating fp32 in PSUM.
- Per core: cache the x-slice KxM [4096,1024] in SBUF (16.8 MB), stream W
  in [128,512] tiles, 8 PSUM banks accumulate the 8 row-blocks over K,
  bias added on DVE during PSUM eviction.
"""

import numpy as np

import concourse.bacc as bacc
import concourse.mybir as mybir
import concourse.tile as tile
from concourse.bass_utils import run_bass_kernel_spmd

N_CORES = 8
N_FULL = 8192  # input rows
K_DIM = 4096  # contraction (in features)
O_DIM = 4096  # out features
M = N_FULL // N_CORES  # rows per core (1024)
P = 128
KO = K_DIM // P  # 32 k-tiles
N_TILE = 512  # moving free dim per matmul (1 PSUM bank of fp32)
N_BLOCKS = O_DIM // N_TILE  # 8
M_BLOCKS = M // P  # 8

_nc_cache = None


def _build():
    nc = bacc.Bacc(target_bir_lowering=False)

    xT = nc.dram_tensor("xT", [K_DIM, M], mybir.dt.float32r, kind="ExternalInput")
    wT = nc.dram_tensor("wT", [K_DIM, O_DIM], mybir.dt.float32r, kind="ExternalInput")
    biasr = nc.dram_tensor("biasr", [P, O_DIM], mybir.dt.float32, kind="ExternalInput")
    y = nc.dram_tensor("y", [M, O_DIM], mybir.dt.float32, kind="ExternalOutput")

    xT3 = xT.ap().rearrange("(ko p) m -> p ko m", p=P)
    wT3 = wT.ap().rearrange("(ko p) n -> p ko n", p=P)
    y3 = y.ap().rearrange("(mb p) n -> p mb n", p=P)

    with tile.TileContext(nc) as tc:
        with (
            tc.tile_pool(name="persist", bufs=1) as persist,
            tc.tile_pool(name="wpool", bufs=10) as wpool,
            tc.tile_pool(name="opool", bufs=10) as opool,
            tc.tile_pool(name="psum", bufs=1, space="PSUM") as psum_pool,
        ):
            # x cached in SBUF, one tile per k-slab so matmuls can start as
            # soon as their slab has landed. Bias comes in per-n-block chunks:
            # chunk 0 early (the first PSUM drain gates on it), the rest after
            # x is done - all on the gpsimd queue, off the w critical path.
            # First two slabs arrive in fine-grained chunks on two queues so
            # the first matmuls start ~6us earlier (single-DMA transfer
            # latency for a full 512KB slab is ~7us).
            x_sb = []
            bias_sb = [None] * N_BLOCKS
            x_chunks = {0: 4, 1: 2}  # ko -> number of load chunks
            for ko in range(KO):
                x_t = persist.tile([P, M], mybir.dt.float32r, tag=f"x{ko}")
                nchunk = x_chunks.get(ko, 1)
                csz = M // nchunk
                for c in range(nchunk):
                    xeng = nc.gpsimd if c % 2 == 0 else nc.scalar
                    xeng.dma_start(
                        x_t[:, c * csz : (c + 1) * csz],
                        xT3[:, ko, c * csz : (c + 1) * csz],
                    )
                x_sb.append(x_t)
                if ko == 0:
                    b_t = persist.tile([P, N_TILE], mybir.dt.float32, tag="bias0")
                    nc.gpsimd.dma_start(b_t[:], biasr.ap()[:, :N_TILE])
                    bias_sb[0] = b_t
            for nb in range(1, N_BLOCKS):
                b_t = persist.tile([P, N_TILE], mybir.dt.float32, tag=f"bias{nb}")
                nc.gpsimd.dma_start(
                    b_t[:], biasr.ap()[:, nb * N_TILE : (nb + 1) * N_TILE]
                )
                bias_sb[nb] = b_t

            for nb in range(N_BLOCKS):
                psums = [
                    psum_pool.tile(
                        [P, N_TILE], mybir.dt.float32, tag=f"ps{m}", name=f"ps{m}"
                    )
                    for m in range(M_BLOCKS)
                ]
                for ko in range(KO):
                    w_t = wpool.tile([P, N_TILE], mybir.dt.float32r, tag="w")
                    weng = nc.sync if ko % 2 == 0 else nc.scalar
                    weng.dma_start(
                        w_t[:], wT3[:, ko, nb * N_TILE : (nb + 1) * N_TILE]
                    )
                    for mb in range(M_BLOCKS):
                        nc.tensor.matmul(
                            psums[mb][:],
                            x_sb[ko][:, mb * P : (mb + 1) * P],
                            w_t[:],
                            start=(ko == 0),
                            stop=(ko == KO - 1),
                        )
                for mb in range(M_BLOCKS):
                    o_t = opool.tile([P, N_TILE], mybir.dt.float32, tag="o")
                    nc.any.tensor_add(o_t[:], psums[mb][:], bias_sb[nb][:])
                    if nb == N_BLOCKS - 1:
                        # w streams are done; use the idle sync/scalar queues
                        # so the tail flush isn't serialized behind gpsimd.
                        oeng = nc.sync if mb % 2 == 0 else nc.scalar
                    else:
                        oeng = nc.gpsimd if mb % 2 == 0 else nc.scalar
                    oeng.dma_start(y3[:, mb, nb * N_TILE : (nb + 1) * N_TILE], o_t[:])

    nc.compile()
    return nc


def _get_nc():
    global _nc_cache
    if _nc_cache is None:
        _nc_cache = _build()
    return _nc_cache


def _make_in_maps(inputs, weights, bias):
    x = np.ascontiguousarray(np.asarray(inputs, dtype=np.float32))
    w = np.ascontiguousarray(np.asarray(weights, dtype=np.float32))
    b = np.asarray(bias, dtype=np.float32)

    xT = x.T  # [K, N_FULL] view
    wT = np.ascontiguousarray(w.T)  # [K, O]
    br = np.ascontiguousarray(np.broadcast_to(b[None, :], (P, O_DIM)))

    in_maps = []
    for c in range(N_CORES):
        xTc = np.ascontiguousarray(xT[:, c * M : (c + 1) * M])
        in_maps.append({"xT": xTc, "wT": wT, "biasr": br})
    return in_maps


def kernel(**inputs):
    nc = _get_nc()
    in_maps = _make_in_maps(inputs["inputs"], inputs["weights"], inputs["bias"])
    res = run_bass_kernel_spmd(nc, in_maps, core_ids=list(range(N_CORES)))
    return np.concatenate([r["y"] for r in res.results], axis=0)


def run_traced(inputs, weights, bias, **trace_kwargs):
    """Used by test.py: same computation, returns (output, BassKernelResults)."""
    nc = _get_nc()
    in_maps = _make_in_maps(inputs, weights, bias)
    res = run_bass_kernel_spmd(
        nc, in_maps, core_ids=list(range(N_CORES)), trace=True, **trace_kwargs
    )
    out = np.concatenate([r["y"] for r in res.results], axis=0)
    return out, res



# revision 4
# speedup vs baseline: 1.0536x; 1.0536x over previous
"""Trainium2 Bass kernel for y = inputs @ weights.T + bias.

Shapes: inputs [8192, 4096] f32, weights [4096, 4096] f32, bias [4096] f32,
output [8192, 4096] f32.

Strategy (v2):
- Data-parallel across 8 NeuronCores: each core computes 1024 rows of the
  output; weights/bias are replicated.
- bf16 compute: host rounds x and w to bf16 (matmul rel err ~3e-3, far under
  the 2e-2 gate). Same PE rate as f32r (1 cycle/row) but half the DMA
  traffic (w alone is 67MB/core in f32) and FWL fast weight loads.
- Transposed product: stationary = w tile [128k, 128o], moving = x
  [128k, 512m], PSUM tile = [128 o-partitions, 512 m]. Bias is then a
  per-partition scalar [128,1], so PSUM eviction runs on BOTH the Scalar
  (ACT activation bias add) and Vector engines, halving drain chains.
- Output stored as yT [4096, 1024] bf16 per core; host transposes/upcasts.
- Phase 1 (x still streaming in): process ob 0-3 together, ko-outer, so
  each x slab is consumed the moment it lands (all 8 PSUM banks).
- Phase 2 (x resident): ob-sequential, PSUM bank pairs cycle with 4-ob
  pipelining depth.
- PE warm-up: dummy matmuls on a zeroed SBUF tile fill the DMA lead-in so
  the HAM clock gate is at 2.4 GHz when real matmuls start.
"""

import numpy as np
import ml_dtypes

import concourse.bacc as bacc
import concourse.mybir as mybir
import concourse.tile as tile
from concourse.bass_utils import run_bass_kernel_spmd

N_CORES = 8
N_FULL = 8192  # input rows
K_DIM = 4096  # contraction (in features)
O_DIM = 4096  # out features
M = N_FULL // N_CORES  # rows per core (1024)
P = 128
KO = K_DIM // P  # 32 k-slabs
OB = O_DIM // P  # 32 output-row blocks
N_TILE = 512  # moving free dim per matmul (1 PSUM bank of fp32)
MC = M // N_TILE  # 2 moving chunks per core
PH1_OBS = 4  # obs processed ko-outer while x streams in
N_DUMMY = 48  # warm-up matmuls (256-wide) to cover the DMA lead-in

_nc_cache = None


def _build():
    nc = bacc.Bacc(target_bir_lowering=False)

    xT = nc.dram_tensor("xT", [K_DIM, M], mybir.dt.bfloat16, kind="ExternalInput")
    wT = nc.dram_tensor("wT", [K_DIM, O_DIM], mybir.dt.bfloat16, kind="ExternalInput")
    biasT = nc.dram_tensor("biasT", [P, OB], mybir.dt.float32, kind="ExternalInput")
    yT = nc.dram_tensor("yT", [O_DIM, M], mybir.dt.bfloat16, kind="ExternalOutput")

    xT3 = xT.ap().rearrange("(ko p) m -> p ko m", p=P)
    wT3 = wT.ap().rearrange("(ko p) o -> p ko o", p=P)
    yT3 = yT.ap().rearrange("(ob p) m -> p ob m", p=P)

    with tile.TileContext(nc) as tc:
        with (
            tc.tile_pool(name="persist", bufs=1) as persist,
            tc.tile_pool(name="wpool", bufs=8) as wpool,
            tc.tile_pool(name="opool", bufs=10) as opool,
            tc.tile_pool(name="psum", bufs=1, space="PSUM") as psum_pool,
        ):
            # --- PE warm-up: dummies on a zeroed tile, result never read.
            dummy_sb = persist.tile([P, 384], mybir.dt.bfloat16, tag="dummy")
            nc.gpsimd.memset(dummy_sb[:], 0)
            # Dummy PSUM shares bank tag ps7: its writes finish long before
            # the first real user of ps7 (phase-1 ob3/mc1) issues.
            dummy_ps = psum_pool.tile([P, N_TILE], mybir.dt.float32, tag="ps7")
            for _ in range(N_DUMMY):
                nc.tensor.matmul(
                    dummy_ps[:, :256],
                    dummy_sb[:, :128],
                    dummy_sb[:, 128:384],
                    start=True,
                    stop=True,
                )

            # --- bias [128, 32] f32
            bias_sb = persist.tile([P, OB], mybir.dt.float32, tag="bias")
            nc.gpsimd.dma_start(bias_sb[:], biasT.ap()[:])

            # --- x preload: 32 slabs [128, 1024] bf16 (256KB each).
            # ko0 lands in two 512-col chunks so the first matmul only waits
            # for 128KB.
            x_sb = []
            for ko in range(KO):
                x_t = persist.tile([P, M], mybir.dt.bfloat16, tag=f"x{ko}")
                nchunk = 2 if ko < 2 else 1
                csz = M // nchunk
                for c in range(nchunk):
                    xeng = nc.gpsimd if (ko + c) % 2 == 0 else nc.scalar
                    xeng.dma_start(
                        x_t[:, c * csz : (c + 1) * csz],
                        xT3[:, ko, c * csz : (c + 1) * csz],
                    )
                x_sb.append(x_t)

            # --- w stream: one tile per ob, [128, 32ko, 128o] bf16 (1MB).
            # ob0 split by ko so MM(ko=0) waits only for 32KB.
            def load_w(ob):
                w_t = wpool.tile([P, KO, P], mybir.dt.bfloat16, tag="w", name="w_t")
                if ob == 0:
                    splits = [(0, 1), (1, 8), (8, KO)]
                elif ob == 1:
                    splits = [(0, 8), (8, KO)]
                else:
                    splits = [(0, KO)]
                for lo, hi in splits:
                    nc.sync.dma_start(
                        w_t[:, lo:hi, :], wT3[:, lo:hi, ob * P : (ob + 1) * P]
                    )
                return w_t

            w_tiles = {}
            for ob in range(PH1_OBS):
                w_tiles[ob] = load_w(ob)

            def evict(ps_t, ob, mc, eng_i):
                o_t = opool.tile([P, N_TILE], mybir.dt.bfloat16, tag="o", name="o_t")
                if eng_i % 2 == 0:
                    nc.scalar.add(o_t[:], ps_t[:], bias_sb[:, ob : ob + 1])
                else:
                    nc.vector.tensor_scalar_add(o_t[:], ps_t[:], bias_sb[:, ob : ob + 1])
                oeng = nc.sync if eng_i % 2 == 0 else nc.gpsimd
                oeng.dma_start(yT3[:, ob, mc * N_TILE : (mc + 1) * N_TILE], o_t[:])

            # --- Phase 1: obs 0..3 ko-outer (8 PSUM banks), consuming each x
            # slab as it lands.
            ps1 = {
                (ob, mc): psum_pool.tile(
                    [P, N_TILE],
                    mybir.dt.float32,
                    tag=f"ps{2 * ob + mc}",
                    name=f"ps{2 * ob + mc}",
                )
                for ob in range(PH1_OBS)
                for mc in range(MC)
            }
            for ko in range(KO):
                for ob in range(PH1_OBS):
                    for mc in range(MC):
                        nc.tensor.matmul(
                            ps1[(ob, mc)][:],
                            w_tiles[ob][:, ko, :],
                            x_sb[ko][:, mc * N_TILE : (mc + 1) * N_TILE],
                            start=(ko == 0),
                            stop=(ko == KO - 1),
                        )
            # prefetch w for the next obs before the eviction burst
            for ob in range(PH1_OBS, 2 * PH1_OBS):
                w_tiles[ob] = load_w(ob)
            for ob in range(PH1_OBS):
                for mc in range(MC):
                    evict(ps1[(ob, mc)], ob, mc, 2 * ob + mc)

            # --- Phase 2: remaining obs sequential, bank pairs cycle mod 4.
            for ob in range(PH1_OBS, OB):
                if ob not in w_tiles:
                    w_tiles[ob] = load_w(ob)
                if ob + 1 < OB and (ob + 1) not in w_tiles:
                    w_tiles[ob + 1] = load_w(ob + 1)
                ps = [
                    psum_pool.tile(
                        [P, N_TILE],
                        mybir.dt.float32,
                        tag=f"ps{2 * (ob % PH1_OBS) + mc}",
                        name=f"ps{2 * (ob % PH1_OBS) + mc}",
                    )
                    for mc in range(MC)
                ]
                for ko in range(KO):
                    for mc in range(MC):
                        nc.tensor.matmul(
                            ps[mc][:],
                            w_tiles[ob][:, ko, :],
                            x_sb[ko][:, mc * N_TILE : (mc + 1) * N_TILE],
                            start=(ko == 0),
                            stop=(ko == KO - 1),
                        )
                del w_tiles[ob]
                for mc in range(MC):
                    evict(ps[mc], ob, mc, mc)

    nc.compile()
    return nc


def _get_nc():
    global _nc_cache
    if _nc_cache is None:
        _nc_cache = _build()
    return _nc_cache


def _make_in_maps(inputs, weights, bias):
    x = np.asarray(inputs, dtype=np.float32)
    w = np.asarray(weights, dtype=np.float32)
    b = np.asarray(bias, dtype=np.float32)

    xT = np.ascontiguousarray(x.T).astype(ml_dtypes.bfloat16)  # [K, N_FULL]
    wT = np.ascontiguousarray(w.T).astype(ml_dtypes.bfloat16)  # [K, O]
    bT = np.ascontiguousarray(b.reshape(OB, P).T)  # [128, 32]

    in_maps = []
    for c in range(N_CORES):
        xTc = np.ascontiguousarray(xT[:, c * M : (c + 1) * M])
        in_maps.append({"xT": xTc, "wT": wT, "biasT": bT})
    return in_maps


def _assemble(res):
    outs = []
    for r in res.results:
        yTc = np.asarray(r["yT"])  # [O, M] bf16
        outs.append(yTc.astype(np.float32).T)  # [M, O] f32
    return np.ascontiguousarray(np.concatenate(outs, axis=0))


def kernel(**inputs):
    nc = _get_nc()
    in_maps = _make_in_maps(inputs["inputs"], inputs["weights"], inputs["bias"])
    res = run_bass_kernel_spmd(nc, in_maps, core_ids=list(range(N_CORES)))
    return _assemble(res)


def run_traced(inputs, weights, bias, **trace_kwargs):
    """Used by test.py: same computation, returns (output, BassKernelResults)."""
    nc = _get_nc()
    in_maps = _make_in_maps(inputs, weights, bias)
    res = run_bass_kernel_spmd(
        nc, in_maps, core_ids=list(range(N_CORES)), trace=True, **trace_kwargs
    )
    return _assemble(res), res
